# revision 39
# baseline (speedup 1.0000x reference)
"""Trainium2 Bass kernel for nn_FeatureRefinement.

Reference computation (bs=16, vl=1024, ql=64, d=1024):
    corr = einsum('bqd,bvd->bqv', Q, V); scores = softmax(corr, axis=1)
    corr_matrix = einsum('bqv,qd->bvd', scores, cor_w)     # cor_w constant over q
    sentence    = WeightedPool(Q)                           # (bs, d)
    sim         = cosine(V, sentence) + log(video_mask)     # (bs, vl)
    features    = concat([V, sim*sim_w, sentence_bcast, corr_matrix], -1)
    out         = relu(features @ mixer_w + mixer_b)

Algebraic restructuring (exact up to fp rounding):
  - softmax over q sums to 1  =>  corr_matrix[b,v,:] == cor_v_w*cor_q_w  (constant)
  - sim_features @ W2  == sim[b,v] * (sim_w.T @ W2)        (rank-1)
  - pooled_query @ W3  == sentence[b] @ W3                 (rank-1 per batch)
  so   out = relu(V @ W1 + [sim; 1; 1]^T @ [w2v; bias_hi; bias_lo])
  The only heavy compute is V @ W1 (4x FLOP reduction) plus O(bs*vl*d)
  vector work for the cosine similarity.

Sharding: data-parallel over batch, 2 batches per core on 8 cores. No
collectives; host scatters inputs / gathers outputs.

Implementation notes:
  - Query side runs in fp16; alpha = Q @ pool_w is one fused DVE op
    against a partition-broadcast pool_w row (no Q^T transposes).
  - Bias rows for both batches are computed in one M=2 matmul group and
    bounced through a DRAM scratch tile into the 3-partition augment rhs
    (engines cannot address partition offsets 1-2 directly).
  - A short stream of junk matmuls at t=0 warms the PE HAM clock gate
    (a cold PE runs at 1.2 GHz for its first ~3.4us of activity).
  - Output is stored fp16 and cast to fp32 on host (well within 2e-2).
  - DMA queue budget (per-queue, not per-link, is the constraint):
    sync carries V (4 MiB @ ~133 GB/s), gpsimd carries W1 (2 MiB @ ~173),
    scalar carries W3 + small tensors early and the fp16 stores late.
"""
import sys

sys.path.insert(0, "/opt/trn_rl_repo")

import numpy as np
import ml_dtypes
from contextlib import ExitStack

import concourse.bass as bass
import concourse.tile as tile
from concourse import bacc, mybir
from concourse.bass_utils import run_bass_kernel_spmd
from concourse.masks import make_identity


def _install_ntff_shim():
    """This container's antenv lacks axon_hooks; if tracing is requested
    (BASS_TRACE=1), run_bass_kernel_spmd would crash importing it. Provide
    the hook via trn_agent_boot's ctypes helper, and keep the trace
    post-processing local (no bucket uploads)."""
    import types
    try:
        import antenv  # noqa: F401
        import antenv.axon_hooks  # noqa: F401
        return  # already present
    except ImportError:
        pass
    try:
        import trn_agent_boot.trn_boot as _tb
        hook = _tb._ntff_profile_via_ctypes("/opt/axon/libaxon_pjrt.so")
        mod = types.ModuleType("antenv.axon_hooks")
        mod.get_axon_ntff_profile_hook = lambda: hook
        sys.modules["antenv.axon_hooks"] = mod
        from concourse import bass_utils as _bu
        _orig = _bu.upload_artifacts

        def _safe_upload(tmpdir):
            try:
                return _orig(tmpdir)
            except Exception:
                return f"file://{tmpdir}"

        _bu.upload_artifacts = _safe_upload
    except Exception:
        pass


_install_ntff_shim()

F32 = mybir.dt.float32
F16 = mybir.dt.float16
BF16 = mybir.dt.bfloat16
AF = mybir.ActivationFunctionType
AX = mybir.AxisListType
ALU = mybir.AluOpType

BS, VL, QL, D = 16, 1024, 64, 1024
NCORES = 8
BPC = BS // NCORES          # batches per core
KC = D // 128               # contraction chunks
SS = 512                    # v-rows per super-slab
NSS = VL // SS              # super-slabs per batch
NEG_INF = -1e30

VDT = F16                   # dtype of the heavy V @ W1 path


def _build_program():
    nc = bacc.Bacc("TRN2", target_bir_lowering=False, debug=False, num_devices=NCORES)

    v_d = nc.dram_tensor("v", [BPC, VL, D], VDT, kind="ExternalInput").ap()
    q_d = nc.dram_tensor("q", [BPC, QL, D], F16, kind="ExternalInput").ap()
    w1_d = nc.dram_tensor("w1", [128, KC, D], VDT, kind="ExternalInput").ap()
    w3_d = nc.dram_tensor("w3", [128, KC, D], VDT, kind="ExternalInput").ap()
    # pool_w pre-broadcast to QL partitions (device partition_broadcast
    # costs a ~14us gpsimd custom-op library load; so does make_identity's
    # iota, hence the identity matrix is an input too)
    pw_d = nc.dram_tensor("pw", [QL, D], F16, kind="ExternalInput").ap()
    biasc2_d = nc.dram_tensor("biasc2", [BPC, D], F32, kind="ExternalInput").ap()
    ident_d = nc.dram_tensor("ident", [128, 128], F32, kind="ExternalInput").ap()
    # packed smalls (one DMA): [qb(2*64) | vb(2*1024, i-interleaved) | w2v(1024)]
    packA_d = nc.dram_tensor("packA", [1, 3200], F32, kind="ExternalInput").ap()
    out_d = nc.dram_tensor("out", [BPC, VL, D], F16, kind="ExternalOutput").ap()

    with tile.TileContext(nc) as tc, ExitStack() as ctx:
        singles = ctx.enter_context(tc.tile_pool(name="singles", bufs=1))
        qstuff = ctx.enter_context(tc.tile_pool(name="qstuff", bufs=1))
        rows = ctx.enter_context(tc.tile_pool(name="rows", bufs=2))
        vload = ctx.enter_context(tc.tile_pool(name="vload", bufs=4))
        trashp = ctx.enter_context(tc.tile_pool(name="trashp", bufs=2))
        psA = ctx.enter_context(tc.tile_pool(name="psA", bufs=2, space="PSUM"))
        psOut = ctx.enter_context(tc.tile_pool(name="psOut", bufs=4, space="PSUM"))
        psRow = ctx.enter_context(tc.tile_pool(name="psRow", bufs=2, space="PSUM"))
        dramp = ctx.enter_context(tc.tile_pool(name="dramp", bufs=1, space="DRAM"))

        # ================= t=0 DMA issues ==========================
        # DMA completion semaphores are a shared pool of ~8: more than that
        # many in-flight DMAs serialize in waves. Consolidate transfers.
        # W1/W3 are host-laid-out partition-major so one DMA moves each
        # with 16 KiB per-partition lines.
        # sync: batch-0 V; gpsimd: W1 then batch-1 V; scalar: q/pw/packs,
        # W3, then the fp16 out stores later.
        # V loads: one DMA per 512-row slab, rows interleaved 4-per-partition
        # (partition p holds rows 4p+j, j=0..3) so both this load and the
        # matching out store move 8 KiB per-partition lines (4x fewer
        # packets on the packet-rate-limited queues). The j index plays the
        # role of the i-tile downstream; sim/vb/store use the same order.
        v_slabs = {}   # (b, s) -> [128, 4, D] tile
        for s in range(NSS):
            v_sb = vload.tile([128, 4, D], VDT, tag="v_sb", name=f"v_0_{s}")
            nc.sync.dma_start(
                out=v_sb,
                in_=v_d[0, s * SS:(s + 1) * SS, :].rearrange(
                    "(p j) d -> p j d", j=4))
            v_slabs[(0, s)] = v_sb

        w1_sb = singles.tile([128, KC, D], VDT)
        nc.gpsimd.dma_start(out=w1_sb, in_=w1_d)
        for s in range(NSS):
            v_sb = vload.tile([128, 4, D], VDT, tag="v_sb", name=f"v_1_{s}")
            nc.gpsimd.dma_start(
                out=v_sb,
                in_=v_d[1, s * SS:(s + 1) * SS, :].rearrange(
                    "(p j) d -> p j d", j=4))
            v_slabs[(1, s)] = v_sb

        q_sb2 = qstuff.tile([QL, BPC, D], F16)
        nc.scalar.dma_start(out=q_sb2, in_=q_d.rearrange("b q d -> q b d"))
        pw64 = singles.tile([QL, D], F16)
        nc.scalar.dma_start(out=pw64, in_=pw_d)
        packA = singles.tile([1, 3200], F32)
        nc.scalar.dma_start(out=packA, in_=packA_d)
        w3_sb = singles.tile([128, KC, D], VDT)
        nc.scalar.dma_start(out=w3_sb, in_=w3_d)
        biasc2 = singles.tile([BPC, D], F32)
        nc.scalar.dma_start(out=biasc2, in_=biasc2_d)
        ident = singles.tile([128, 128], F32)
        nc.scalar.dma_start(out=ident, in_=ident_d)

        def qb_row(b):
            return packA[:, b * QL:(b + 1) * QL]

        def vb_row(b, lo, hi):
            return packA[:, 2 * QL + b * VL + lo:2 * QL + b * VL + hi]

        w2v_row = packA[:, 2 * QL + 2 * VL:2 * QL + 2 * VL + D]

        aug3 = [qstuff.tile([3, D], BF16, name=f"aug3_{b}") for b in range(BPC)]
        for b in range(BPC):
            nc.vector.tensor_copy(aug3[b][0:1, :], w2v_row)

        # ================= HAM warmup ==============================
        warm16 = singles.tile([128, 512], F16)
        nc.vector.memset(warm16, 0.0)
        for r in range(12):
            warm_ps = psOut.tile([128, 512], F32, tag="o_ps", name=f"warm{r}")
            nc.tensor.matmul(warm_ps, warm16[:, 0:128], warm16,
                             start=True, stop=True)

        identH = singles.tile([128, 128], VDT)
        nc.vector.tensor_copy(identH, ident)

        # ================= Phase A: query side =====================
        sentT2 = qstuff.tile([128, KC, BPC], VDT)    # sentence^T chunks
        snsq2 = qstuff.tile([1, BPC], F32)           # clamped ||sentence||^2

        for b in range(BPC):
            q_sb = q_sb2[:, b, :]
            # alpha[q] = sum_d Q[q,d]*pw[d]  (one fused DVE op)
            qtrash = trashp.tile([QL, D], F16, tag="qtrash")
            alpha_col = rows.tile([QL, 1], F32)
            nc.vector.scalar_tensor_tensor(
                out=qtrash, in0=q_sb, scalar=1.0, in1=pw64,
                op0=ALU.mult, op1=ALU.mult, accum_out=alpha_col)
            al_ps = psRow.tile([1, QL], F32, tag="row")
            nc.tensor.transpose(al_ps, alpha_col, ident[:QL, :QL])
            alpha_sb = rows.tile([1, QL], F32)
            nc.vector.tensor_add(alpha_sb, al_ps, qb_row(b))

            # softmax over the free dim (1 partition)
            mx = rows.tile([1, 1], F32)
            nc.vector.reduce_max(mx, alpha_sb, axis=AX.X)
            asub = rows.tile([1, QL], F32)
            nc.vector.tensor_scalar_sub(asub, alpha_sb, mx)
            aexp = rows.tile([1, QL], F32)
            asum = rows.tile([1, 1], F32)
            nc.scalar.activation(aexp, asub, AF.Exp, accum_out=asum)
            rsum = rows.tile([1, 1], F32)
            nc.vector.reciprocal(rsum, asum)
            alphas_sb = rows.tile([1, QL], F32)
            nc.vector.tensor_scalar_mul(alphas_sb, aexp, rsum)

            # alphas^T : [QL, 1] fp16 (lhsT of the sentence matmul)
            alT_ps = psRow.tile([QL, 1], F32, tag="row")
            nc.tensor.transpose(alT_ps, alphas_sb, ident[:1, :1])
            alphasT_sb = rows.tile([QL, 1], F16)
            nc.vector.tensor_copy(alphasT_sb, alT_ps)

            # sentence = alphas @ Q : [1, D] fp32
            sent_sb = rows.tile([1, D], F32, tag="sent", bufs=1)
            for h in range(2):
                s_ps = psRow.tile([1, 512], F32, tag="row")
                nc.tensor.matmul(s_ps, alphasT_sb, q_sb[:, h * 512:(h + 1) * 512],
                                 start=True, stop=True)
                nc.vector.tensor_copy(sent_sb[:, h * 512:(h + 1) * 512], s_ps)

            # ||sentence||^2 clamped
            strash = rows.tile([1, D], F32, tag="strash", bufs=1)
            ssq = rows.tile([1, 1], F32)
            nc.scalar.activation(strash, sent_sb, AF.Square, accum_out=ssq)
            nc.vector.tensor_scalar_max(snsq2[:, b:b + 1], ssq, 1e-16)

            # sentence^T chunks: sentT2[p,k] = sent[k*128+p]
            sT_ps = psRow.tile([128, KC], F32, tag="row")
            for k in range(KC):
                nc.tensor.transpose(sT_ps[:, k:k + 1],
                                    sent_sb[:, k * 128:(k + 1) * 128],
                                    ident[:1, :1])
            nc.vector.tensor_copy(sentT2[:, :, b], sT_ps)

        # augment lhsT tiles: rows 1:3 are the constant ones
        aug_l = [[qstuff.tile([3, SS], BF16, name=f"augl_{b}_{s}")
                  for s in range(NSS)] for b in range(BPC)]
        for b in range(BPC):
            for s in range(NSS):
                nc.gpsimd.memset(aug_l[b][s], 1.0)  # row 0 overwritten by sim

        def emit_bias_rows():
            # bias rows, both batches at once (M=2):
            #   bias_f[b] = sentence[b] @ W3 + biasc, split bf16 hi+lo
            bias_f = rows.tile([2, D], F32, tag="biasf", bufs=1)
            for h in range(2):
                b_ps = psRow.tile([2, 512], F32, tag="row")
                for k in range(KC):
                    nc.tensor.matmul(b_ps, sentT2[:, k, 0:BPC],
                                     w3_sb[:, k, h * 512:(h + 1) * 512],
                                     start=(k == 0), stop=(k == KC - 1))
                nc.vector.tensor_add(bias_f[:, h * 512:(h + 1) * 512], b_ps,
                                     biasc2[:, h * 512:(h + 1) * 512])
            bias_hi = rows.tile([2, D], BF16, tag="biashi", bufs=1)
            nc.vector.tensor_copy(bias_hi, bias_f)
            bias_lo = rows.tile([2, D], BF16, tag="biaslo", bufs=1)
            nc.vector.tensor_sub(bias_lo, bias_f, bias_hi)
            # engines can't write partitions 1:3 of aug3 directly; bounce the
            # bias rows through a DRAM scratch tile (DMA has no such limit)
            augd = dramp.tile([BPC, 2, D], BF16)
            nc.gpsimd.dma_start(out=augd[:, 0, :], in_=bias_hi)
            nc.gpsimd.dma_start(out=augd[:, 1, :], in_=bias_lo)
            for b in range(BPC):
                nc.gpsimd.dma_start(out=aug3[b][1:3, :], in_=augd[b])

        # ================= Phase C: video side (heavy) =============
        # Per-slab C1 (load+norm+transpose) immediately followed by that
        # slab's C2 (matmuls): the PE engine queue is in-order, so emitting
        # work whose inputs arrive late would head-of-line block it.
        vtpool = ctx.enter_context(tc.tile_pool(name="vtpool", bufs=4))
        opool = ctx.enter_context(tc.tile_pool(name="opool", bufs=2))

        for b in range(BPC):
            for s in range(NSS):
                # --- C1: row norms + transpose into vt
                vt = vtpool.tile([128, KC, SS], VDT, tag="vt", name=f"vt_{b}_{s}")
                vnsq_col = rows.tile([128, 4], F32, tag="vnsqc")
                for s4 in range(4):
                    v_sb = v_slabs[(b, s)][:, s4, :]
                    vtrash = trashp.tile([128, D], F32, tag="vtrash")
                    nc.scalar.activation(vtrash, v_sb, AF.Square,
                                         accum_out=vnsq_col[:, s4:s4 + 1])
                    for g in range(2):
                        t_ps = psA.tile([128, 512], VDT, tag="tps")
                        for j in range(4):
                            k = g * 4 + j
                            nc.tensor.transpose(
                                t_ps[:, j * 128:(j + 1) * 128],
                                v_sb[:, k * 128:(k + 1) * 128], identH)
                        nc.vector.tensor_copy(
                            vt[:, g * 4:(g + 1) * 4, s4 * 128:(s4 + 1) * 128],
                            t_ps.rearrange("p (j c) -> p j c", j=4))

                if b == 0 and s == 0:
                    emit_bias_rows()

                # --- C2: sim row + main matmuls
                # dot row: sentence . V^T  -> [1, SS]
                dot_ps = psRow.tile([1, SS], F32, tag="row")
                for k in range(KC):
                    nc.tensor.matmul(dot_ps, sentT2[:, k, b:b + 1], vt[:, k, :],
                                     start=(k == 0), stop=(k == KC - 1))
                vnr_ps = psRow.tile([1, SS], F32, tag="row")
                for s4 in range(4):
                    nc.tensor.transpose(vnr_ps[:, s4 * 128:(s4 + 1) * 128],
                                        vnsq_col[:, s4:s4 + 1], ident)

                # sim = dot * rsqrt(max(vnsq,eps)*snsq) + log(video_mask)
                t1 = rows.tile([1, SS], F32, tag="t1")
                nc.vector.tensor_scalar(t1, vnr_ps, 1e-16, snsq2[:, b:b + 1],
                                        op0=ALU.max, op1=ALU.mult)
                t3 = rows.tile([1, SS], F32, tag="t3")
                nc.scalar.activation(t3, t1, AF.Abs_reciprocal_sqrt)
                t4 = rows.tile([1, SS], F32, tag="t4")
                nc.vector.tensor_mul(t4, dot_ps, t3)
                nc.vector.tensor_add(aug_l[b][s][0:1, :], t4,
                                     vb_row(b, s * SS, (s + 1) * SS))

                out_sb = opool.tile([128, 4, D], F16)  # whole slab, 1 store
                for i in range(4):
                    o_ps = [psOut.tile([128, 512], F32, tag="o_ps",
                                       name=f"o_ps_{b}_{s}_{i}_{h}")
                            for h in range(2)]
                    # keep 8 consecutive MMs on one PSUM bank: per-instruction
                    # bank alternation triggers the PE depth-cycling penalty
                    for h in range(2):
                        for k in range(KC):
                            nc.tensor.matmul(
                                o_ps[h], vt[:, k, i * 128:(i + 1) * 128],
                                w1_sb[:, k, h * 512:(h + 1) * 512],
                                start=(k == 0), stop=False)
                    for h in range(2):
                        nc.tensor.matmul(
                            o_ps[h], aug_l[b][s][:, i * 128:(i + 1) * 128],
                            aug3[b][:, h * 512:(h + 1) * 512],
                            start=False, stop=True)
                        # relu on DVE (fp16 store)
                        nc.vector.tensor_scalar_max(
                            out_sb[:, i, h * 512:(h + 1) * 512], o_ps[h], 0.0)
                nc.scalar.dma_start(
                    out=out_d[b, s * SS:(s + 1) * SS, :].rearrange(
                        "(p j) d -> p j d", j=4),
                    in_=out_sb)

    nc.compile()
    return nc


_NC = None
_LAST_RESULTS = None


def _get_program():
    global _NC
    if _NC is None:
        _NC = _build_program()
    return _NC


def kernel(video_features, query_features, video_mask, query_mask,
           sim_w, cor_v_w, cor_q_w, pool_w, mixer_w, mixer_b):
    video_features = np.asarray(video_features, dtype=np.float32)
    query_features = np.asarray(query_features, dtype=np.float32)
    video_mask = np.asarray(video_mask, dtype=np.float32)
    query_mask = np.asarray(query_mask, dtype=np.float32)
    sim_w = np.asarray(sim_w, dtype=np.float32)
    cor_v_w = np.asarray(cor_v_w, dtype=np.float32)
    cor_q_w = np.asarray(cor_q_w, dtype=np.float32)
    pool_w = np.asarray(pool_w, dtype=np.float32)
    mixer_w = np.asarray(mixer_w, dtype=np.float32)
    mixer_b = np.asarray(mixer_b, dtype=np.float32)

    # host-side folds of the weight-only algebra (O(d^2), negligible).
    # W1/W3 in partition-major layout w[p, k, n] = W[k*128+p, n] so the
    # whole tensor loads as one DMA with 16 KiB per-partition lines.
    W1p = np.ascontiguousarray(
        mixer_w[0:D].reshape(KC, 128, D).transpose(1, 0, 2)).astype(np.float16)
    W2 = mixer_w[D:2 * D]
    W3p = np.ascontiguousarray(
        mixer_w[2 * D:3 * D].reshape(KC, 128, D).transpose(1, 0, 2)).astype(np.float16)
    W4 = mixer_w[3 * D:4 * D]
    w2v = (sim_w[:, 0] @ W2.astype(np.float32)).astype(np.float32)
    cor_vec = (cor_v_w[0] * cor_q_w[0, 0]).astype(np.float32)
    biasc = (cor_vec @ W4 + mixer_b).astype(np.float32)
    biasc2 = np.ascontiguousarray(np.broadcast_to(biasc, (BPC, D)))
    qbias = ((1.0 - query_mask) * NEG_INF).astype(np.float32)
    vbias = np.log(video_mask + 1e-45).astype(np.float32)
    # vb in the device's interleaved order: slab position j*128+p <-> row 4p+j
    vbias_il = np.ascontiguousarray(
        vbias.reshape(BS, NSS, 128, 4).transpose(0, 1, 3, 2).reshape(BS, VL))
    pw64 = np.ascontiguousarray(
        np.broadcast_to(pool_w[:, 0], (QL, D))).astype(np.float16)
    identity = np.eye(128, dtype=np.float32)
    v16 = video_features.astype(np.float16)
    q16 = query_features.astype(np.float16)

    nc = _get_program()
    in_maps = []
    for c in range(NCORES):
        sl = slice(c * BPC, (c + 1) * BPC)
        packA = np.concatenate(
            [qbias[sl].reshape(-1), vbias_il[sl].reshape(-1), w2v])[None, :]
        in_maps.append({
            "v": np.ascontiguousarray(v16[sl]),
            "q": np.ascontiguousarray(q16[sl]),
            "w1": W1p,
            "w3": W3p,
            "pw": pw64,
            "biasc2": biasc2,
            "ident": identity,
            "packA": np.ascontiguousarray(packA),
        })
    res = run_bass_kernel_spmd(nc, in_maps, core_ids=list(range(NCORES)))
    global _LAST_RESULTS
    _LAST_RESULTS = res
    out = np.concatenate([res.results[c]["out"] for c in range(NCORES)], axis=0)
    return out.astype(np.float32)


# revision 43
# speedup vs baseline: 1.0222x; 1.0222x over previous
"""Trainium2 Bass kernel for nn_FeatureRefinement.

Reference computation (bs=16, vl=1024, ql=64, d=1024):
    corr = einsum('bqd,bvd->bqv', Q, V); scores = softmax(corr, axis=1)
    corr_matrix = einsum('bqv,qd->bvd', scores, cor_w)     # cor_w constant over q
    sentence    = WeightedPool(Q)                           # (bs, d)
    sim         = cosine(V, sentence) + log(video_mask)     # (bs, vl)
    features    = concat([V, sim*sim_w, sentence_bcast, corr_matrix], -1)
    out         = relu(features @ mixer_w + mixer_b)

Algebraic restructuring (exact up to fp rounding):
  - softmax over q sums to 1  =>  corr_matrix[b,v,:] == cor_v_w*cor_q_w  (constant)
  - sim_features @ W2  == sim[b,v] * (sim_w.T @ W2)        (rank-1)
  - pooled_query @ W3  == sentence[b] @ W3                 (rank-1 per batch)
  so   out = relu(V @ W1 + [sim; 1; 1]^T @ [w2v; bias_hi; bias_lo])
  The only heavy compute is V @ W1 (4x FLOP reduction) plus O(bs*vl*d)
  vector work for the cosine similarity.

Sharding: data-parallel over batch, 2 batches per core on 8 cores. No
collectives; host scatters inputs / gathers outputs.

Implementation notes:
  - Query side runs in fp16; alpha = Q @ pool_w is one fused DVE op
    against a partition-broadcast pool_w row (no Q^T transposes).
  - Bias rows for both batches are computed in one M=2 matmul group and
    bounced through a DRAM scratch tile into the 3-partition augment rhs
    (engines cannot address partition offsets 1-2 directly).
  - A short stream of junk matmuls at t=0 warms the PE HAM clock gate
    (a cold PE runs at 1.2 GHz for its first ~3.4us of activity).
  - Output is stored fp16 and cast to fp32 on host (well within 2e-2).
  - DMA queue budget (per-queue, not per-link, is the constraint):
    sync carries V (4 MiB @ ~133 GB/s), gpsimd carries W1 (2 MiB @ ~173),
    scalar carries W3 + small tensors early and the fp16 stores late.
"""
import sys

sys.path.insert(0, "/opt/trn_rl_repo")

import numpy as np
import ml_dtypes
from contextlib import ExitStack

import concourse.bass as bass
import concourse.tile as tile
from concourse import bacc, mybir
from concourse.bass_utils import run_bass_kernel_spmd
from concourse.masks import make_identity


def _install_ntff_shim():
    """This container's antenv lacks axon_hooks; if tracing is requested
    (BASS_TRACE=1), run_bass_kernel_spmd would crash importing it. Provide
    the hook via trn_agent_boot's ctypes helper, and keep the trace
    post-processing local (no bucket uploads)."""
    import types
    try:
        import antenv  # noqa: F401
        import antenv.axon_hooks  # noqa: F401
        return  # already present
    except ImportError:
        pass
    try:
        import trn_agent_boot.trn_boot as _tb
        hook = _tb._ntff_profile_via_ctypes("/opt/axon/libaxon_pjrt.so")
        mod = types.ModuleType("antenv.axon_hooks")
        mod.get_axon_ntff_profile_hook = lambda: hook
        sys.modules["antenv.axon_hooks"] = mod
        from concourse import bass_utils as _bu
        _orig = _bu.upload_artifacts

        def _safe_upload(tmpdir):
            try:
                return _orig(tmpdir)
            except Exception:
                return f"file://{tmpdir}"

        _bu.upload_artifacts = _safe_upload
    except Exception:
        pass


_install_ntff_shim()

F32 = mybir.dt.float32
F16 = mybir.dt.float16
BF16 = mybir.dt.bfloat16
AF = mybir.ActivationFunctionType
AX = mybir.AxisListType
ALU = mybir.AluOpType

BS, VL, QL, D = 16, 1024, 64, 1024
NCORES = 8
BPC = BS // NCORES          # batches per core
KC = D // 128               # contraction chunks
SS = 512                    # v-rows per super-slab
NSS = VL // SS              # super-slabs per batch
NEG_INF = -1e30

VDT = F16                   # dtype of the heavy V @ W1 path


def _build_program():
    nc = bacc.Bacc("TRN2", target_bir_lowering=False, debug=False, num_devices=NCORES)

    v_d = nc.dram_tensor("v", [BPC, VL, D], VDT, kind="ExternalInput").ap()
    q_d = nc.dram_tensor("q", [BPC, QL, D], F16, kind="ExternalInput").ap()
    w1_d = nc.dram_tensor("w1", [2, 128, KC, 512], VDT, kind="ExternalInput").ap()
    w3_d = nc.dram_tensor("w3", [128, KC, D], VDT, kind="ExternalInput").ap()
    # pool_w pre-broadcast to QL partitions (device partition_broadcast
    # costs a ~14us gpsimd custom-op library load; so does make_identity's
    # iota, hence the identity matrix is an input too)
    pw_d = nc.dram_tensor("pw", [QL, D], F16, kind="ExternalInput").ap()
    biasc2_d = nc.dram_tensor("biasc2", [BPC, D], F32, kind="ExternalInput").ap()
    ident_d = nc.dram_tensor("ident", [128, 128], F32, kind="ExternalInput").ap()
    # packed smalls (one DMA): [qb(2*64) | vb(2*1024, i-interleaved) | w2v(1024)]
    packA_d = nc.dram_tensor("packA", [1, 3200], F32, kind="ExternalInput").ap()
    out_d = nc.dram_tensor("out", [BPC, VL, D], F16, kind="ExternalOutput").ap()

    with tile.TileContext(nc) as tc, ExitStack() as ctx:
        singles = ctx.enter_context(tc.tile_pool(name="singles", bufs=1))
        qstuff = ctx.enter_context(tc.tile_pool(name="qstuff", bufs=1))
        rows = ctx.enter_context(tc.tile_pool(name="rows", bufs=2))
        vload = ctx.enter_context(tc.tile_pool(name="vload", bufs=4))
        trashp = ctx.enter_context(tc.tile_pool(name="trashp", bufs=2))
        psA = ctx.enter_context(tc.tile_pool(name="psA", bufs=2, space="PSUM"))
        psOut = ctx.enter_context(tc.tile_pool(name="psOut", bufs=4, space="PSUM"))
        psRow = ctx.enter_context(tc.tile_pool(name="psRow", bufs=2, space="PSUM"))
        dramp = ctx.enter_context(tc.tile_pool(name="dramp", bufs=1, space="DRAM"))

        # ================= t=0 DMA issues ==========================
        # DMA completion semaphores are a shared pool of ~8: more than that
        # many in-flight DMAs serialize in waves. Consolidate transfers.
        # W1/W3 are host-laid-out partition-major so one DMA moves each
        # with 16 KiB per-partition lines.
        # sync: batch-0 V; gpsimd: W1 then batch-1 V; scalar: q/pw/packs,
        # W3, then the fp16 out stores later.
        # V rows interleaved 4-per-partition (partition p holds rows 4p+j,
        # j=0..3) so loads and the matching out stores move 4-8 KiB
        # per-partition lines. The j index plays the role of the i-tile
        # downstream; sim/vb/store use the same order.
        # Queue plan (~134 GB/s each; deadlines drive placement):
        #   scalar: ident/q/pw/smalls, W1-h0, then the out stores
        #   sync:   V b0s0 halves, W1-h1, V b0s1 halves
        #   gpsimd (starts ~5us late): W3, V b1, bias bounce
        ident = singles.tile([128, 128], F32)
        nc.scalar.dma_start(out=ident, in_=ident_d)
        q_sb2 = qstuff.tile([QL, BPC, D], F16)
        nc.scalar.dma_start(out=q_sb2, in_=q_d.rearrange("b q d -> q b d"))
        pw64 = singles.tile([QL, D], F16)
        nc.scalar.dma_start(out=pw64, in_=pw_d)
        packA = singles.tile([1, 3200], F32)
        nc.scalar.dma_start(out=packA, in_=packA_d)
        biasc2 = singles.tile([BPC, D], F32)
        nc.scalar.dma_start(out=biasc2, in_=biasc2_d)
        w1_sb = singles.tile([128, 2, KC, 512], VDT)  # h-major
        nc.scalar.dma_start(out=w1_sb[:, 0], in_=w1_d[0])

        v_slabs = {}   # (b, s) -> [128, 4, D] tile
        for s in range(NSS):
            v_slabs[(0, s)] = vload.tile([128, 4, D], VDT, tag="v_sb",
                                         name=f"v_0_{s}")
        for half in range(2):  # b0s0 halves first, then W1-h1, then b0s1
            nc.sync.dma_start(
                out=v_slabs[(0, 0)][:, 2 * half:2 * half + 2, :],
                in_=v_d[0, 0:SS, :].rearrange(
                    "(p j) d -> p j d", j=4)[:, 2 * half:2 * half + 2, :])
        nc.sync.dma_start(out=w1_sb[:, 1], in_=w1_d[1])
        for half in range(2):
            nc.sync.dma_start(
                out=v_slabs[(0, 1)][:, 2 * half:2 * half + 2, :],
                in_=v_d[0, SS:2 * SS, :].rearrange(
                    "(p j) d -> p j d", j=4)[:, 2 * half:2 * half + 2, :])

        w3_sb = singles.tile([128, KC, D], VDT)
        nc.gpsimd.dma_start(out=w3_sb, in_=w3_d)
        for s in range(NSS):
            v_sb = vload.tile([128, 4, D], VDT, tag="v_sb", name=f"v_1_{s}")
            nc.gpsimd.dma_start(
                out=v_sb,
                in_=v_d[1, s * SS:(s + 1) * SS, :].rearrange(
                    "(p j) d -> p j d", j=4))
            v_slabs[(1, s)] = v_sb

        def qb_row(b):
            return packA[:, b * QL:(b + 1) * QL]

        def vb_row(b, lo, hi):
            return packA[:, 2 * QL + b * VL + lo:2 * QL + b * VL + hi]

        w2v_row = packA[:, 2 * QL + 2 * VL:2 * QL + 2 * VL + D]

        aug3 = [qstuff.tile([3, D], BF16, name=f"aug3_{b}") for b in range(BPC)]
        for b in range(BPC):
            nc.vector.tensor_copy(aug3[b][0:1, :], w2v_row)

        # ================= HAM warmup ==============================
        warm16 = singles.tile([128, 512], F16)
        nc.vector.memset(warm16, 0.0)
        for r in range(12):
            warm_ps = psOut.tile([128, 512], F32, tag="o_ps", name=f"warm{r}")
            nc.tensor.matmul(warm_ps, warm16[:, 0:128], warm16,
                             start=True, stop=True)

        identH = singles.tile([128, 128], VDT)
        nc.vector.tensor_copy(identH, ident)

        # ================= Phase A: query side =====================
        sentT2 = qstuff.tile([128, KC, BPC], VDT)    # sentence^T chunks
        snsq2 = qstuff.tile([1, BPC], F32)           # clamped ||sentence||^2

        for b in range(BPC):
            q_sb = q_sb2[:, b, :]
            # alpha[q] = sum_d Q[q,d]*pw[d]  (one fused DVE op)
            qtrash = trashp.tile([QL, D], F16, tag="qtrash")
            alpha_col = rows.tile([QL, 1], F32)
            nc.vector.scalar_tensor_tensor(
                out=qtrash, in0=q_sb, scalar=1.0, in1=pw64,
                op0=ALU.mult, op1=ALU.mult, accum_out=alpha_col)
            al_ps = psRow.tile([1, QL], F32, tag="row")
            nc.tensor.transpose(al_ps, alpha_col, ident[:QL, :QL])
            alpha_sb = rows.tile([1, QL], F32)
            nc.vector.tensor_add(alpha_sb, al_ps, qb_row(b))

            # softmax over the free dim (1 partition)
            mx = rows.tile([1, 1], F32)
            nc.vector.reduce_max(mx, alpha_sb, axis=AX.X)
            asub = rows.tile([1, QL], F32)
            nc.vector.tensor_scalar_sub(asub, alpha_sb, mx)
            aexp = rows.tile([1, QL], F32)
            asum = rows.tile([1, 1], F32)
            nc.scalar.activation(aexp, asub, AF.Exp, accum_out=asum)
            rsum = rows.tile([1, 1], F32)
            nc.vector.reciprocal(rsum, asum)
            alphas_sb = rows.tile([1, QL], F32)
            nc.vector.tensor_scalar_mul(alphas_sb, aexp, rsum)

            # alphas^T : [QL, 1] fp16 (lhsT of the sentence matmul)
            alT_ps = psRow.tile([QL, 1], F32, tag="row")
            nc.tensor.transpose(alT_ps, alphas_sb, ident[:1, :1])
            alphasT_sb = rows.tile([QL, 1], F16)
            nc.vector.tensor_copy(alphasT_sb, alT_ps)

            # sentence = alphas @ Q : [1, D] fp32
            sent_sb = rows.tile([1, D], F32, tag="sent", bufs=1)
            for h in range(2):
                s_ps = psRow.tile([1, 512], F32, tag="row")
                nc.tensor.matmul(s_ps, alphasT_sb, q_sb[:, h * 512:(h + 1) * 512],
                                 start=True, stop=True)
                nc.vector.tensor_copy(sent_sb[:, h * 512:(h + 1) * 512], s_ps)

            # ||sentence||^2 clamped
            strash = rows.tile([1, D], F32, tag="strash", bufs=1)
            ssq = rows.tile([1, 1], F32)
            nc.scalar.activation(strash, sent_sb, AF.Square, accum_out=ssq)
            nc.vector.tensor_scalar_max(snsq2[:, b:b + 1], ssq, 1e-16)

            # sentence^T chunks: sentT2[p,k] = sent[k*128+p]
            sT_ps = psRow.tile([128, KC], F32, tag="row")
            for k in range(KC):
                nc.tensor.transpose(sT_ps[:, k:k + 1],
                                    sent_sb[:, k * 128:(k + 1) * 128],
                                    ident[:1, :1])
            nc.vector.tensor_copy(sentT2[:, :, b], sT_ps)

        # augment lhsT tiles: rows 1:3 are the constant ones
        aug_l = [[qstuff.tile([3, SS], BF16, name=f"augl_{b}_{s}")
                  for s in range(NSS)] for b in range(BPC)]
        for b in range(BPC):
            for s in range(NSS):
                nc.gpsimd.memset(aug_l[b][s], 1.0)  # row 0 overwritten by sim

        def emit_bias_rows():
            # bias rows, both batches at once (M=2):
            #   bias_f[b] = sentence[b] @ W3 + biasc, split bf16 hi+lo
            bias_f = rows.tile([2, D], F32, tag="biasf", bufs=1)
            for h in range(2):
                b_ps = psRow.tile([2, 512], F32, tag="row")
                for k in range(KC):
                    nc.tensor.matmul(b_ps, sentT2[:, k, 0:BPC],
                                     w3_sb[:, k, h * 512:(h + 1) * 512],
                                     start=(k == 0), stop=(k == KC - 1))
                nc.vector.tensor_add(bias_f[:, h * 512:(h + 1) * 512], b_ps,
                                     biasc2[:, h * 512:(h + 1) * 512])
            bias_hi = rows.tile([2, D], BF16, tag="biashi", bufs=1)
            nc.vector.tensor_copy(bias_hi, bias_f)
            bias_lo = rows.tile([2, D], BF16, tag="biaslo", bufs=1)
            nc.vector.tensor_sub(bias_lo, bias_f, bias_hi)
            # engines can't write partitions 1:3 of aug3 directly; bounce the
            # bias rows through a DRAM scratch tile (DMA has no such limit)
            augd = dramp.tile([BPC, 2, D], BF16)
            nc.gpsimd.dma_start(out=augd[:, 0, :], in_=bias_hi)
            nc.gpsimd.dma_start(out=augd[:, 1, :], in_=bias_lo)
            for b in range(BPC):
                nc.gpsimd.dma_start(out=aug3[b][1:3, :], in_=augd[b])

        # ================= Phase C: video side (heavy) =============
        # Per-slab C1 (load+norm+transpose) immediately followed by that
        # slab's C2 (matmuls): the PE engine queue is in-order, so emitting
        # work whose inputs arrive late would head-of-line block it.
        vtpool = ctx.enter_context(tc.tile_pool(name="vtpool", bufs=4))
        opool = ctx.enter_context(tc.tile_pool(name="opool", bufs=2))

        for b in range(BPC):
            for s in range(NSS):
                # --- C1: row norms + transpose into vt
                vt = vtpool.tile([128, KC, SS], VDT, tag="vt", name=f"vt_{b}_{s}")
                vnsq_col = rows.tile([128, 4], F32, tag="vnsqc")
                for s4 in range(4):
                    v_sb = v_slabs[(b, s)][:, s4, :]
                    vtrash = trashp.tile([128, D], F32, tag="vtrash")
                    nc.scalar.activation(vtrash, v_sb, AF.Square,
                                         accum_out=vnsq_col[:, s4:s4 + 1])
                    for g in range(2):
                        t_ps = psA.tile([128, 512], VDT, tag="tps")
                        for j in range(4):
                            k = g * 4 + j
                            nc.tensor.transpose(
                                t_ps[:, j * 128:(j + 1) * 128],
                                v_sb[:, k * 128:(k + 1) * 128], identH)
                        nc.vector.tensor_copy(
                            vt[:, g * 4:(g + 1) * 4, s4 * 128:(s4 + 1) * 128],
                            t_ps.rearrange("p (j c) -> p j c", j=4))

                if b == 0 and s == 0:
                    emit_bias_rows()

                # --- C2: sim row + main matmuls
                # dot row: sentence . V^T  -> [1, SS]
                dot_ps = psRow.tile([1, SS], F32, tag="row")
                for k in range(KC):
                    nc.tensor.matmul(dot_ps, sentT2[:, k, b:b + 1], vt[:, k, :],
                                     start=(k == 0), stop=(k == KC - 1))
                vnr_ps = psRow.tile([1, SS], F32, tag="row")
                for s4 in range(4):
                    nc.tensor.transpose(vnr_ps[:, s4 * 128:(s4 + 1) * 128],
                                        vnsq_col[:, s4:s4 + 1], ident)

                # sim = dot * rsqrt(max(vnsq,eps)*snsq) + log(video_mask)
                t1 = rows.tile([1, SS], F32, tag="t1")
                nc.vector.tensor_scalar(t1, vnr_ps, 1e-16, snsq2[:, b:b + 1],
                                        op0=ALU.max, op1=ALU.mult)
                t3 = rows.tile([1, SS], F32, tag="t3")
                nc.scalar.activation(t3, t1, AF.Abs_reciprocal_sqrt)
                t4 = rows.tile([1, SS], F32, tag="t4")
                nc.vector.tensor_mul(t4, dot_ps, t3)
                nc.vector.tensor_add(aug_l[b][s][0:1, :], t4,
                                     vb_row(b, s * SS, (s + 1) * SS))

                out_sb = opool.tile([128, 4, D], F16)  # whole slab, 1 store
                for i in range(4):
                    o_ps = [psOut.tile([128, 512], F32, tag="o_ps",
                                       name=f"o_ps_{b}_{s}_{i}_{h}")
                            for h in range(2)]
                    # keep 8 consecutive MMs on one PSUM bank: per-instruction
                    # bank alternation triggers the PE depth-cycling penalty
                    for h in range(2):
                        for k in range(KC):
                            nc.tensor.matmul(
                                o_ps[h], vt[:, k, i * 128:(i + 1) * 128],
                                w1_sb[:, h, k, :],
                                start=(k == 0), stop=False)
                    for h in range(2):
                        nc.tensor.matmul(
                            o_ps[h], aug_l[b][s][:, i * 128:(i + 1) * 128],
                            aug3[b][:, h * 512:(h + 1) * 512],
                            start=False, stop=True)
                        # relu on DVE (fp16 store)
                        nc.vector.tensor_scalar_max(
                            out_sb[:, i, h * 512:(h + 1) * 512], o_ps[h], 0.0)
                nc.scalar.dma_start(
                    out=out_d[b, s * SS:(s + 1) * SS, :].rearrange(
                        "(p j) d -> p j d", j=4),
                    in_=out_sb)

    nc.compile()
    return nc


_NC = None
_LAST_RESULTS = None


def _get_program():
    global _NC
    if _NC is None:
        _NC = _build_program()
    return _NC


def kernel(video_features, query_features, video_mask, query_mask,
           sim_w, cor_v_w, cor_q_w, pool_w, mixer_w, mixer_b):
    video_features = np.asarray(video_features, dtype=np.float32)
    query_features = np.asarray(query_features, dtype=np.float32)
    video_mask = np.asarray(video_mask, dtype=np.float32)
    query_mask = np.asarray(query_mask, dtype=np.float32)
    sim_w = np.asarray(sim_w, dtype=np.float32)
    cor_v_w = np.asarray(cor_v_w, dtype=np.float32)
    cor_q_w = np.asarray(cor_q_w, dtype=np.float32)
    pool_w = np.asarray(pool_w, dtype=np.float32)
    mixer_w = np.asarray(mixer_w, dtype=np.float32)
    mixer_b = np.asarray(mixer_b, dtype=np.float32)

    # host-side folds of the weight-only algebra (O(d^2), negligible).
    # W1 in h-major partition-major layout w1[h, p, k, n'] = W1[k*128+p,
    # h*512+n'] (two 1-MiB DMAs, 8 KiB lines); W3 partition-major
    # w3[p, k, n] = W3[k*128+p, n] (one DMA, 16 KiB lines).
    W1p = np.ascontiguousarray(
        mixer_w[0:D].reshape(KC, 128, 2, 512).transpose(2, 1, 0, 3)).astype(np.float16)
    W2 = mixer_w[D:2 * D]
    W3p = np.ascontiguousarray(
        mixer_w[2 * D:3 * D].reshape(KC, 128, D).transpose(1, 0, 2)).astype(np.float16)
    W4 = mixer_w[3 * D:4 * D]
    w2v = (sim_w[:, 0] @ W2.astype(np.float32)).astype(np.float32)
    cor_vec = (cor_v_w[0] * cor_q_w[0, 0]).astype(np.float32)
    biasc = (cor_vec @ W4 + mixer_b).astype(np.float32)
    biasc2 = np.ascontiguousarray(np.broadcast_to(biasc, (BPC, D)))
    qbias = ((1.0 - query_mask) * NEG_INF).astype(np.float32)
    vbias = np.log(video_mask + 1e-45).astype(np.float32)
    # vb in the device's interleaved order: slab position j*128+p <-> row 4p+j
    vbias_il = np.ascontiguousarray(
        vbias.reshape(BS, NSS, 128, 4).transpose(0, 1, 3, 2).reshape(BS, VL))
    pw64 = np.ascontiguousarray(
        np.broadcast_to(pool_w[:, 0], (QL, D))).astype(np.float16)
    identity = np.eye(128, dtype=np.float32)
    v16 = video_features.astype(np.float16)
    q16 = query_features.astype(np.float16)

    nc = _get_program()
    in_maps = []
    for c in range(NCORES):
        sl = slice(c * BPC, (c + 1) * BPC)
        packA = np.concatenate(
            [qbias[sl].reshape(-1), vbias_il[sl].reshape(-1), w2v])[None, :]
        in_maps.append({
            "v": np.ascontiguousarray(v16[sl]),
            "q": np.ascontiguousarray(q16[sl]),
            "w1": W1p,
            "w3": W3p,
            "pw": pw64,
            "biasc2": biasc2,
            "ident": identity,
            "packA": np.ascontiguousarray(packA),
        })
    res = run_bass_kernel_spmd(nc, in_maps, core_ids=list(range(NCORES)))
    global _LAST_RESULTS
    _LAST_RESULTS = res
    out = np.concatenate([res.results[c]["out"] for c in range(NCORES)], axis=0)
    return out.astype(np.float32)
